# revision 1
# baseline (speedup 1.0000x reference)
"""Causal multi-head attention Trainium2 kernel (8 NeuronCores).

Problem: B=4, L=2048, D=1024, 16 heads x (dh=64, dv=64), causal mask.
Sharding: data-parallel over batch (4) x tensor-parallel over heads (2 groups
of 8). Core c handles batch c//2, head-group c%2. Each core computes its
partial output projection (ctx_g @ Wo_g); the host sums the two head-group
partials per batch and adds the bias.

v2: software-pipelined. The prologue transposes x (PE) into a resident
xT [d, l] f32r tile, computes V = x@Wv (+ones column for the fused softmax
denominator) and Q^T/K^T for head-pair 0. The main loop runs flash-style
attention per head-pair while dribbling the next head-pair's Q^T/K^T
projection matmuls between attention groups — keeping the PE dense so the
HAM clock gate stays at 2.4 GHz. All matmuls run in float32r (TF32 rate).
S^T tiles = K@Q^T; exp on ACT (scale=1/8 folded in); causal diagonal via one
tril tensor_mul per diag k-tile + width-restricted PV; ones column of V_aug
gives the softmax denominator in PSUM row 64; normalize with DVE
reciprocal_approx_fast + gpsimd partition_broadcast + DVE multiply into the
resident ctx^T tile; output projection at the end.
"""

import numpy as np
from contextlib import ExitStack

import concourse.bass as bass
import concourse.tile as tile
from concourse import bacc, mybir
from concourse.masks import make_identity

F32 = mybir.dt.float32
F32R = mybir.dt.float32r
AF = mybir.ActivationFunctionType

B, L, D = 4, 2048, 1024
N_HEAD, DH, DV = 16, 64, 64
N_CORES = 8
HPC = N_HEAD // 2          # heads per core (8)
OC = HPC * DH              # per-core projection width (512)
NHP = HPC // 2             # head-pairs per core (4)


class ProjEmitter:
    """Q^T/K^T projection for one head-pair, emitted in per-(proj,chunk)
    units so the matmuls interleave with attention of the previous pair."""

    def __init__(self, nc, hp, pools, xt, wq, wk, nch):
        self.nc = nc
        self.xt = xt
        qkp, wp, self.psP = pools
        self.wq_sb = wp.tile([128, 8, 128], F32R, tag="wq")
        self.wk_sb = wp.tile([128, 8, 128], F32R, tag="wk")
        nc.sync.dma_start(
            out=self.wq_sb,
            in_=wq[:, hp * 128:(hp + 1) * 128].rearrange("(t p) o -> p t o", p=128))
        nc.sync.dma_start(
            out=self.wk_sb,
            in_=wk[:, hp * 128:(hp + 1) * 128].rearrange("(t p) o -> p t o", p=128))
        self.qt = qkp.tile([128, nch * 512], F32R, tag="qt")
        self.kt = qkp.tile([128, nch * 512], F32R, tag="kt")
        self.units = [(w, d, c) for w, d in ((self.wq_sb, self.qt),
                                             (self.wk_sb, self.kt))
                      for c in range(nch)]
        self.i = 0

    def step(self):
        if self.i >= len(self.units):
            return False
        w_sb, dst, c = self.units[self.i]
        self.i += 1
        nc = self.nc
        pp = self.psP.tile([128, 512], F32, tag="pp")
        for d in range(8):
            nc.tensor.matmul(pp, w_sb[:, d, :],
                             self.xt[:, d, c * 512:(c + 1) * 512],
                             start=(d == 0), stop=(d == 7))
        nc.vector.tensor_copy(dst[:, c * 512:(c + 1) * 512], pp)
        return True

    def drain(self):
        while self.step():
            pass


def build_nc(l=L):
    assert l % 512 == 0
    nch = l // 512           # q-chunks
    nlt = l // 128           # l-tiles
    nc = bacc.Bacc("TRN2", target_bir_lowering=False, debug=False,
                   num_devices=N_CORES)

    x = nc.dram_tensor("x", [l, D], F32, kind="ExternalInput").ap()
    wq = nc.dram_tensor("wq", [D, OC], F32R, kind="ExternalInput").ap()
    wk = nc.dram_tensor("wk", [D, OC], F32R, kind="ExternalInput").ap()
    wv = nc.dram_tensor("wv", [D, OC], F32R, kind="ExternalInput").ap()
    wo = nc.dram_tensor("wo", [OC, D], F32R, kind="ExternalInput").ap()
    out = nc.dram_tensor("out", [l, D], F32, kind="ExternalOutput").ap()

    with tile.TileContext(nc) as tc, ExitStack() as ctx:
        top = ctx.enter_context(tc.tile_pool(name="top", bufs=1))
        psP = ctx.enter_context(tc.tile_pool(name="psP", bufs=2, space="PSUM"))
        xtp = ctx.enter_context(tc.tile_pool(name="xtp", bufs=1))
        qkp = ctx.enter_context(tc.tile_pool(name="qkp", bufs=2))
        wp = ctx.enter_context(tc.tile_pool(name="wp", bufs=2))

        # V: [128(l), ltile, head, 65] - col 64 is ones (softmax denominator)
        vt = top.tile([128, nlt, HPC, DH + 1], F32R)
        ct = top.tile([128, NHP, l], F32R)        # normalized ctx^T
        tril = top.tile([128, 128], F32)
        ones = top.tile([128, 1], F32)
        xt = xtp.tile([128, 8, l], F32R)          # x^T, d-major

        nc.vector.memset(ones, 1.0)
        nc.vector.tensor_copy(
            vt[:, :, :, DV:DV + 1].rearrange("p t h c -> p (t h) c"),
            ones.broadcast_to((128, nlt * HPC, 1)))
        # causal keep-mask for S^T diag blocks: tril[k, q] = 1.0 iff q >= k
        nc.gpsimd.memset(tril, 0.0)
        nc.gpsimd.affine_select(
            out=tril, in_=tril, compare_op=mybir.AluOpType.is_gt,
            fill=1.0, base=0, pattern=[[-1, 128]], channel_multiplier=1)

        # ---------------- Prologue: transpose + V + QK(hp=0) --------------
        with tc.tile_pool(name="pro", bufs=3) as pro, \
             tc.tile_pool(name="wvp", bufs=1) as wvp, \
             tc.tile_pool(name="psT", bufs=3, space="PSUM") as psT:
            ident = wvp.tile([128, 128], F32)
            make_identity(nc, ident)
            wv_sb = wvp.tile([128, 8, OC], F32R)
            nc.sync.dma_start(out=wv_sb,
                              in_=wv.rearrange("(t p) o -> p t o", p=128))
            for c in range(nch):
                for s in range(4):
                    xst = pro.tile([128, D], F32, tag="xst")
                    nc.sync.dma_start(
                        out=xst,
                        in_=x[c * 512 + s * 128: c * 512 + (s + 1) * 128, :])
                    for d in range(8):
                        pt = psT.tile([128, 128], F32, tag="pt")
                        nc.tensor.transpose(
                            pt, xst[:, d * 128:(d + 1) * 128], ident)
                        nc.vector.tensor_copy(
                            xt[:, d, c * 512 + s * 128: c * 512 + (s + 1) * 128],
                            pt)
                # V for this l-chunk
                for m in range(4):
                    pp = psP.tile([128, 512], F32, tag="pp")
                    for d in range(8):
                        nc.tensor.matmul(
                            pp, xt[:, d, (c * 4 + m) * 128:(c * 4 + m + 1) * 128],
                            wv_sb[:, d, :], start=(d == 0), stop=(d == 7))
                    nc.vector.tensor_copy(
                        vt[:, c * 4 + m, :, 0:DV],
                        pp.rearrange("p (h v) -> p h v", h=HPC))
            em = ProjEmitter(nc, 0, (qkp, wp, psP), xt, wq, wk, nch)
            em.drain()

        # ---------------- Main: attention + next-pair projections ---------
        with tc.tile_pool(name="phb", bufs=2) as phb, \
             tc.tile_pool(name="psS", bufs=2, space="PSUM") as psS, \
             tc.tile_pool(name="psC", bufs=2, space="PSUM") as psC:
            n_groups_hp = 2 * nch * (nch + 1)
            for hp in range(NHP):
                qt, kt = em.qt, em.kt
                em = (ProjEmitter(nc, hp + 1, (qkp, wp, psP), xt, wq, wk, nch)
                      if hp + 1 < NHP else None)
                cadence = max(1, (n_groups_hp // 2) // (2 * nch)) if em else 0
                gcount = 0

                def pv_step(g, j, pctx, pexp, po, H):
                    # masks + PV for group g (one group after its exp)
                    for r2 in range(2):
                        kt_i = 2 * g + r2
                        r = kt_i - 4 * j
                        c0 = 0
                        if r >= 0:      # diagonal k-tile
                            c0 = r * 128
                            nc.vector.tensor_mul(
                                pexp[:, r2, c0:c0 + 128],
                                pexp[:, r2, c0:c0 + 128], tril)
                        nc.tensor.matmul(
                            pctx[:, c0:512],
                            vt[:, kt_i, H, :],
                            pexp[:, r2, c0:512],
                            start=(kt_i == 0), stop=(kt_i == 4 * j + 3))

                # both heads interleaved at group level: two independent
                # dependency chains keep the PE busy through each other's
                # exp waits
                for j in range(nch):
                    n_g = 2 * (j + 1)
                    pctxs = {}
                    prevs = {0: None, 1: None}
                    for h in range(2):
                        pctxs[h] = psC.tile([DV + 1, 512], F32,
                                            tag="pctx", name=f"pctx{h}")
                    for g in range(n_g + 1):
                        for h in range(2):
                            po = 64 * h
                            H = 2 * hp + h
                            pexp = None
                            if g < n_g:
                                psc = psS.tile([128, 2, 512], F32,
                                               tag="psc", name=f"psc{h}")
                                for r2 in range(2):
                                    kt_i = 2 * g + r2
                                    nc.tensor.matmul(
                                        psc[:, r2, :],
                                        kt[po:po + DH,
                                           kt_i * 128:(kt_i + 1) * 128],
                                        qt[po:po + DH, j * 512:(j + 1) * 512],
                                        start=True, stop=True)
                                pexp = phb.tile([128, 2, 512], F32R,
                                                tag="pexp", bufs=4,
                                                name=f"pexp{h}")
                                nc.scalar.activation(pexp, psc, AF.Exp,
                                                     scale=0.125)
                            if prevs[h] is not None:
                                pv_step(prevs[h][0], j, pctxs[h],
                                        prevs[h][1], po, H)
                                gcount += 1
                                did = False
                                if em and cadence and gcount % cadence == 0:
                                    did = em.step()
                                if not did:
                                    # space heater: keeps HAM at 2.4GHz
                                    ppd = psP.tile([128, 512], F32,
                                                   tag="pp", name="ppd")
                                    nc.tensor.matmul(
                                        ppd[0:DV, :], vt[:, 0, 0, 0:DV],
                                        qt[:, 0:512], start=True, stop=True)
                            prevs[h] = (g, pexp) if g < n_g else None
                    for h in range(2):
                        po = 64 * h
                        rs = phb.tile([1, 512], F32, tag="rs", name="rs")
                        nc.vector.tensor_copy(rs, pctxs[h][DV:DV + 1, :])
                        inv = phb.tile([1, 512], F32, tag="inv", name="inv")
                        nc.vector.reciprocal_approx_fast(out=inv, in_=rs)
                        bc = phb.tile([64, 512], F32, tag="bc", name="bc")
                        nc.gpsimd.partition_broadcast(out_ap=bc, in_ap=inv)
                        nc.vector.tensor_mul(
                            ct[po:po + DV, hp, j * 512:(j + 1) * 512],
                            pctxs[h][0:DV, :], bc)
                if em:
                    em.drain()

        # ---------------- Output projection ----------------
        with tc.tile_pool(name="phc", bufs=1) as phc, \
             tc.tile_pool(name="phco", bufs=3) as phco:
            wo_sb = phc.tile([128, 4, D], F32R)
            nc.sync.dma_start(out=wo_sb,
                              in_=wo.rearrange("(t p) o -> p t o", p=128))
            for lt in range(nlt):
                ost = phco.tile([128, D], F32, tag="ost")
                for n in range(2):
                    pp = psP.tile([128, 512], F32, tag="pp")
                    for v in range(4):
                        nc.tensor.matmul(
                            pp, ct[:, v, lt * 128:(lt + 1) * 128],
                            wo_sb[:, v, n * 512:(n + 1) * 512],
                            start=(v == 0), stop=(v == 3))
                    nc.vector.tensor_copy(ost[:, n * 512:(n + 1) * 512], pp)
                nc.sync.dma_start(out=out[lt * 128:(lt + 1) * 128, :], in_=ost)

    nc.compile()
    return nc


def tf32_round(x):
    """Round fp32 -> tf32 (10-bit mantissa)."""
    xi = np.ascontiguousarray(x).view(np.uint32)
    return ((xi + 0x1000) & 0xFFFFE000).view(np.float32)


def make_in_maps(x, Wq, Wk, Wv, Wo):
    in_maps = []
    for c in range(N_CORES):
        b, g = c // 2, c % 2
        in_maps.append({
            "x": np.ascontiguousarray(x[b]),
            "wq": tf32_round(Wq[:, g * OC:(g + 1) * OC]),
            "wk": tf32_round(Wk[:, g * OC:(g + 1) * OC]),
            "wv": tf32_round(Wv[:, g * OC:(g + 1) * OC]),
            "wo": tf32_round(Wo[g * OC:(g + 1) * OC, :]),
        })
    return in_maps


_NC_CACHE = {}


def _get_nc():
    if "nc" not in _NC_CACHE:
        _NC_CACHE["nc"] = build_nc()
    return _NC_CACHE["nc"]


def _numpy_fallback(x, Wq, Wk, Wv, Wo, bo, mask):
    Bsz, Lq, _ = x.shape
    Q = (x @ Wq).reshape(Bsz, Lq, N_HEAD, DH).transpose(0, 2, 1, 3)
    K = (x @ Wk).reshape(Bsz, Lq, N_HEAD, DH).transpose(0, 2, 1, 3)
    V = (x @ Wv).reshape(Bsz, Lq, N_HEAD, DV).transpose(0, 2, 1, 3)
    s = np.einsum("bhqd,bhkd->bhqk", Q, K) / np.sqrt(np.float32(DH))
    s = np.where(mask, s, -np.inf)
    s = s - s.max(axis=-1, keepdims=True)
    p = np.exp(s)
    p /= p.sum(axis=-1, keepdims=True)
    ctxv = np.einsum("bhqk,bhkv->bhqv", p, V)
    ctxv = ctxv.transpose(0, 2, 1, 3).reshape(Bsz, Lq, N_HEAD * DV)
    return (ctxv @ Wo + bo).astype(np.float32)


def run_on_hw(in_maps, trace=False):
    from concourse.bass_utils import run_bass_kernel_spmd
    nc = _get_nc()
    return run_bass_kernel_spmd(nc, in_maps, list(range(N_CORES)), trace=trace)


def kernel(x, Wq, Wk, Wv, Wo, bo, mask, _trace=False, _results=None):
    x = np.asarray(x, dtype=np.float32)
    Wq = np.asarray(Wq, dtype=np.float32)
    Wk = np.asarray(Wk, dtype=np.float32)
    Wv = np.asarray(Wv, dtype=np.float32)
    Wo = np.asarray(Wo, dtype=np.float32)
    bo = np.asarray(bo, dtype=np.float32)
    mask_np = np.asarray(mask).reshape(mask.shape[-2], mask.shape[-1])

    causal = bool(np.array_equal(
        mask_np, np.tril(np.ones((L, L), dtype=bool))))
    if not causal or x.shape != (B, L, D):
        return _numpy_fallback(np.asarray(x), Wq, Wk, Wv, Wo, bo,
                               np.asarray(mask))

    res = run_on_hw(make_in_maps(x, Wq, Wk, Wv, Wo), trace=_trace)
    if _results is not None:
        _results.append(res)
    out = np.empty((B, L, D), dtype=np.float32)
    for b in range(B):
        out[b] = res.results[2 * b]["out"] + res.results[2 * b + 1]["out"] + bo
    return out



# revision 4
# speedup vs baseline: 1.2596x; 1.2596x over previous
"""Causal multi-head attention Trainium2 kernel (8 NeuronCores).

Problem: B=4, L=2048, D=1024, 16 heads x (dh=64, dv=64), causal mask.
Sharding: data-parallel over batch (4) x tensor-parallel over heads (2 groups
of 8). Core c handles batch c//2, head-group c%2. Each core computes its
partial output projection (ctx_g @ Wo_g); the host sums the two head-group
partials per batch and adds the bias.

v3: bf16 everywhere on SBUF (PSUM accumulates f32), x pre-transposed on the
host into [128, 8, L] d-major layout so the kernel has no PE transposes.
Loop order is q-chunk-outer: for each 512-query chunk j the four head-pairs
run flash-style attention (S^T = K@Q^T per 128-k-tile, exp on ACT with the
1/8 scale folded in, tril mask on the diagonal tiles, PV accumulation with
a ones-column of V giving the softmax denominator in PSUM row 64), while
the Q/K/V projections of chunk j+1 and the output projection of earlier
chunks dribble between attention groups to keep the PE dense at 2.4 GHz.
S^T and exp are trimmed to the causal region at 128-column granularity.
"""

import numpy as np
from contextlib import ExitStack

import ml_dtypes

import concourse.bass as bass
import concourse.tile as tile
from concourse import bacc, mybir

F32 = mybir.dt.float32
BF16 = mybir.dt.bfloat16
AF = mybir.ActivationFunctionType

B, L, D = 4, 2048, 1024
N_HEAD, DH, DV = 16, 64, 64
N_CORES = 8
HPC = N_HEAD // 2          # heads per core (8)
OC = HPC * DH              # per-core projection width (512)
NHP = HPC // 2             # head-pairs per core (4)
NCH = L // 512             # q-chunks (4)
NLT = L // 128             # l-tiles (16)


def build_nc():
    nc = bacc.Bacc("TRN2", target_bir_lowering=False, debug=False,
                   num_devices=N_CORES)

    xt = nc.dram_tensor("xt", [128, 8, L], BF16, kind="ExternalInput").ap()
    wq = nc.dram_tensor("wq", [128, 8, OC], BF16, kind="ExternalInput").ap()
    wk = nc.dram_tensor("wk", [128, 8, OC], BF16, kind="ExternalInput").ap()
    wv = nc.dram_tensor("wv", [128, 8, OC], BF16, kind="ExternalInput").ap()
    wo = nc.dram_tensor("wo", [128, 4, D], BF16, kind="ExternalInput").ap()
    out = nc.dram_tensor("out", [L, D], F32, kind="ExternalOutput").ap()

    with tile.TileContext(nc) as tc, ExitStack() as ctx:
        top = ctx.enter_context(tc.tile_pool(name="top", bufs=1))
        psP = ctx.enter_context(tc.tile_pool(name="psP", bufs=2, space="PSUM"))
        psS = ctx.enter_context(tc.tile_pool(name="psS", bufs=2, space="PSUM"))
        psC = ctx.enter_context(tc.tile_pool(name="psC", bufs=2, space="PSUM"))
        phb = ctx.enter_context(tc.tile_pool(name="phb", bufs=2))
        pho = ctx.enter_context(tc.tile_pool(name="pho", bufs=3))

        xts = top.tile([128, 8, L], BF16)
        wqs = top.tile([128, 8, OC], BF16)
        wks = top.tile([128, 8, OC], BF16)
        wvs = top.tile([128, 8, OC], BF16)
        wos = top.tile([128, 4, D], BF16)
        qt = top.tile([128, NHP, L], BF16)
        kt = top.tile([128, NHP, L], BF16)
        # V: [128(k), ltile, head, 65] - col 64 is ones (softmax denominator)
        vt = top.tile([128, NLT, HPC, DV + 1], BF16)
        ct = top.tile([128, NHP, L], BF16)        # normalized ctx^T
        trilf = top.tile([128, 128], F32)
        tril = top.tile([128, 128], BF16)
        ones = top.tile([128, 1], BF16)

        # input DMAs; x chunked so chunk-0 projections can start early
        nc.sync.dma_start(out=wqs, in_=wq)
        nc.sync.dma_start(out=wks, in_=wk)
        nc.sync.dma_start(out=xts[:, :, 0:512], in_=xt[:, :, 0:512])
        nc.sync.dma_start(out=wvs, in_=wv)
        for c in range(1, NCH):
            nc.sync.dma_start(out=xts[:, :, c * 512:(c + 1) * 512],
                              in_=xt[:, :, c * 512:(c + 1) * 512])
        nc.sync.dma_start(out=wos, in_=wo)

        nc.vector.memset(ones, 1.0)
        nc.vector.tensor_copy(
            vt[:, :, :, DV:DV + 1].rearrange("p t h c -> p (t h) c"),
            ones.broadcast_to((128, NLT * HPC, 1)))
        # causal keep-mask for S^T diag blocks: tril[k, q] = 1.0 iff q >= k
        nc.gpsimd.memset(trilf, 0.0)
        nc.gpsimd.affine_select(
            out=trilf, in_=trilf, compare_op=mybir.AluOpType.is_gt,
            fill=1.0, base=0, pattern=[[-1, 128]], channel_multiplier=1)
        nc.vector.tensor_copy(tril, trilf)

        # ---------------- projection / output units ----------------
        def qk_unit(c, hp, wsrc, dst):
            def run():
                pp = psP.tile([128, 512], F32, tag="pp", name="pp")
                for d in range(8):
                    nc.tensor.matmul(pp, wsrc[:, d, hp * 128:(hp + 1) * 128],
                                     xts[:, d, c * 512:(c + 1) * 512],
                                     start=(d == 0), stop=(d == 7))
                nc.vector.tensor_copy(dst[:, hp, c * 512:(c + 1) * 512], pp)
            return run

        def v_unit(lt):
            def run():
                pp = psP.tile([128, 512], F32, tag="pp", name="pp")
                for d in range(8):
                    nc.tensor.matmul(pp, xts[:, d, lt * 128:(lt + 1) * 128],
                                     wvs[:, d, :], start=(d == 0),
                                     stop=(d == 7))
                nc.vector.tensor_copy(
                    vt[:, lt, :, 0:DV],
                    pp.rearrange("p (h v) -> p h v", h=HPC))
            return run

        ost_map = {}

        def o_unit(lt, n):
            def run():
                if n == 0:
                    ost_map[lt] = pho.tile([128, D], F32, tag="ost", name="ost")
                ost = ost_map[lt]
                pp = psP.tile([128, 512], F32, tag="pp", name="pp")
                for v in range(4):
                    nc.tensor.matmul(pp, ct[:, v, lt * 128:(lt + 1) * 128],
                                     wos[:, v, n * 512:(n + 1) * 512],
                                     start=(v == 0), stop=(v == 3))
                nc.vector.tensor_copy(ost[:, n * 512:(n + 1) * 512], pp)
                if n == 1:
                    nc.sync.dma_start(out=out[lt * 128:(lt + 1) * 128, :],
                                      in_=ost)
                    del ost_map[lt]
            return run

        def heater():
            # keeps the HAM clock gate at 2.4GHz when no real unit is ready
            pp = psP.tile([128, 512], F32, tag="pp", name="ppd")
            nc.tensor.matmul(pp[0:DV + 1, :], vt[:, 0, 0, :],
                             qt[:, 0, 0:512], start=True, stop=True)

        # ---------------- attention for one (head-pair, q-chunk) ----------
        def attention(hp, j, units):
            n_g = 2 * (j + 1)
            pctxs = {h: psC.tile([DV + 1, 512], F32, tag="pctx",
                                 name=f"pctx{h}") for h in range(2)}
            prevs = {0: None, 1: None}
            for g in range(n_g + 1):
                for h in range(2):
                    po = 64 * h
                    pexp = None
                    if g < n_g:
                        psc = psS.tile([128, 2, 512], F32, tag="psc",
                                       name=f"psc{h}")
                        pexp = phb.tile([128, 2, 512], BF16, tag="pexp",
                                        bufs=4, name=f"pexp{h}")
                        c0s = []
                        for r2 in range(2):
                            kt_i = 2 * g + r2
                            r = kt_i - 4 * j
                            c0 = 128 * r if r > 0 else 0
                            c0s.append(c0)
                            nc.tensor.matmul(
                                psc[:, r2, c0:512],
                                kt[po:po + DH, hp,
                                   kt_i * 128:(kt_i + 1) * 128],
                                qt[po:po + DH, hp,
                                   j * 512 + c0:(j + 1) * 512],
                                start=True, stop=True)
                        if c0s[0] == c0s[1]:
                            nc.scalar.activation(
                                pexp[:, :, c0s[0]:512], psc[:, :, c0s[0]:512],
                                AF.Exp, scale=0.125)
                        else:
                            for r2 in range(2):
                                nc.scalar.activation(
                                    pexp[:, r2, c0s[r2]:512],
                                    psc[:, r2, c0s[r2]:512],
                                    AF.Exp, scale=0.125)
                        # mask the causal diagonal blocks right after exp
                        for r2 in range(2):
                            r = 2 * g + r2 - 4 * j
                            if r >= 0:
                                nc.vector.tensor_mul(
                                    pexp[:, r2, r * 128:(r + 1) * 128],
                                    pexp[:, r2, r * 128:(r + 1) * 128], tril)
                        if g == 0:
                            # no PV yet; fill the exp-wait bubble
                            if units:
                                units.pop(0)()
                            else:
                                heater()
                    if prevs[h] is not None:
                        pg, ppexp = prevs[h]
                        H = 2 * hp + h
                        for r2 in range(2):
                            kt_i = 2 * pg + r2
                            r = kt_i - 4 * j
                            c0 = 128 * r if r > 0 else 0
                            nc.tensor.matmul(
                                pctxs[h][:, c0:512],
                                vt[:, kt_i, H, :],
                                ppexp[:, r2, c0:512],
                                start=(kt_i == 0), stop=(kt_i == 4 * j + 3))
                        if units:
                            units.pop(0)()
                        else:
                            heater()
                    prevs[h] = (g, pexp) if g < n_g else None
            for h in range(2):
                po = 64 * h
                rs = phb.tile([1, 512], F32, tag="rs", name="rs")
                nc.vector.tensor_copy(rs, pctxs[h][DV:DV + 1, :])
                inv = phb.tile([1, 512], F32, tag="inv", name="inv")
                nc.vector.reciprocal_approx_fast(out=inv, in_=rs)
                bc = phb.tile([64, 512], F32, tag="bc", name="bc")
                nc.gpsimd.partition_broadcast(out_ap=bc, in_ap=inv)
                nc.vector.tensor_mul(
                    ct[po:po + DV, hp, j * 512:(j + 1) * 512],
                    pctxs[h][0:DV, :], bc)

        # ---------------- schedule ----------------
        # prologue: just enough of chunk 0 for attention(hp0) to start
        qk_unit(0, 0, wqs, qt)()
        qk_unit(0, 0, wks, kt)()
        for lt in range(4):
            v_unit(lt)()

        for j in range(NCH):
            units = []
            if j == 0:
                for hp in range(1, NHP):
                    units.append(qk_unit(0, hp, wqs, qt))
                    units.append(qk_unit(0, hp, wks, kt))
            if j + 1 < NCH:
                for hp in range(NHP):
                    units.append(qk_unit(j + 1, hp, wqs, qt))
                    units.append(qk_unit(j + 1, hp, wks, kt))
                for lt in range(4 * (j + 1), 4 * (j + 2)):
                    units.append(v_unit(lt))
            if j >= 1:
                # O(j-1): ct for chunk j-1 is complete
                for lt in range(4 * (j - 1), 4 * j):
                    units.append(o_unit(lt, 0))
                    units.append(o_unit(lt, 1))
            for hp in range(NHP):
                attention(hp, j, units)
            while units:
                units.pop(0)()

        for lt in range(4 * (NCH - 1), 4 * NCH):
            o_unit(lt, 0)()
            o_unit(lt, 1)()

    nc.compile()
    return nc


def make_in_maps(x, Wq, Wk, Wv, Wo):
    bf = ml_dtypes.bfloat16
    in_maps = []
    for c in range(N_CORES):
        b, g = c // 2, c % 2
        xtb = np.ascontiguousarray(
            x[b].T.reshape(8, 128, L).transpose(1, 0, 2)).astype(bf)
        wqg = np.ascontiguousarray(
            Wq[:, g * OC:(g + 1) * OC].reshape(8, 128, OC)
            .transpose(1, 0, 2)).astype(bf)
        wkg = np.ascontiguousarray(
            Wk[:, g * OC:(g + 1) * OC].reshape(8, 128, OC)
            .transpose(1, 0, 2)).astype(bf)
        wvg = np.ascontiguousarray(
            Wv[:, g * OC:(g + 1) * OC].reshape(8, 128, OC)
            .transpose(1, 0, 2)).astype(bf)
        wog = np.ascontiguousarray(
            Wo[g * OC:(g + 1) * OC, :].reshape(4, 128, D)
            .transpose(1, 0, 2)).astype(bf)
        in_maps.append({"xt": xtb, "wq": wqg, "wk": wkg, "wv": wvg,
                        "wo": wog})
    return in_maps


_NC_CACHE = {}


def _get_nc():
    if "nc" not in _NC_CACHE:
        _NC_CACHE["nc"] = build_nc()
    return _NC_CACHE["nc"]


def _numpy_fallback(x, Wq, Wk, Wv, Wo, bo, mask):
    Bsz, Lq, _ = x.shape
    Q = (x @ Wq).reshape(Bsz, Lq, N_HEAD, DH).transpose(0, 2, 1, 3)
    K = (x @ Wk).reshape(Bsz, Lq, N_HEAD, DH).transpose(0, 2, 1, 3)
    V = (x @ Wv).reshape(Bsz, Lq, N_HEAD, DV).transpose(0, 2, 1, 3)
    s = np.einsum("bhqd,bhkd->bhqk", Q, K) / np.sqrt(np.float32(DH))
    s = np.where(mask, s, -np.inf)
    s = s - s.max(axis=-1, keepdims=True)
    p = np.exp(s)
    p /= p.sum(axis=-1, keepdims=True)
    ctxv = np.einsum("bhqk,bhkv->bhqv", p, V)
    ctxv = ctxv.transpose(0, 2, 1, 3).reshape(Bsz, Lq, N_HEAD * DV)
    return (ctxv @ Wo + bo).astype(np.float32)


def run_on_hw(in_maps, trace=False):
    from concourse.bass_utils import run_bass_kernel_spmd
    nc = _get_nc()
    return run_bass_kernel_spmd(nc, in_maps, list(range(N_CORES)),
                                trace=trace)


def kernel(x, Wq, Wk, Wv, Wo, bo, mask, _trace=False, _results=None):
    x = np.asarray(x, dtype=np.float32)
    Wq = np.asarray(Wq, dtype=np.float32)
    Wk = np.asarray(Wk, dtype=np.float32)
    Wv = np.asarray(Wv, dtype=np.float32)
    Wo = np.asarray(Wo, dtype=np.float32)
    bo = np.asarray(bo, dtype=np.float32)
    mask_np = np.asarray(mask).reshape(mask.shape[-2], mask.shape[-1])

    causal = bool(np.array_equal(
        mask_np, np.tril(np.ones((L, L), dtype=bool))))
    if not causal or x.shape != (B, L, D):
        return _numpy_fallback(np.asarray(x), Wq, Wk, Wv, Wo, bo,
                               np.asarray(mask))

    res = run_on_hw(make_in_maps(x, Wq, Wk, Wv, Wo), trace=_trace)
    if _results is not None:
        _results.append(res)
    out = np.empty((B, L, D), dtype=np.float32)
    for b in range(B):
        out[b] = res.results[2 * b]["out"] + res.results[2 * b + 1]["out"] + bo
    return out


# revision 5
# speedup vs baseline: 1.3337x; 1.0589x over previous
"""Causal multi-head attention Trainium2 kernel (8 NeuronCores).

Problem: B=4, L=2048, D=1024, 16 heads x (dh=64, dv=64), causal mask.
Sharding: data-parallel over batch (4) x tensor-parallel over heads (2 groups
of 8). Core c handles batch c//2, head-group c%2. Each core computes its
partial output projection (ctx_g @ Wo_g); the host sums the two head-group
partials per batch and adds the bias.

v3: bf16 everywhere on SBUF (PSUM accumulates f32), x pre-transposed on the
host into [128, 8, L] d-major layout so the kernel has no PE transposes.
Loop order is q-chunk-outer: for each 512-query chunk j the four head-pairs
run flash-style attention (S^T = K@Q^T per 128-k-tile, exp on ACT with the
1/8 scale folded in, tril mask on the diagonal tiles, PV accumulation with
a ones-column of V giving the softmax denominator in PSUM row 64), while
the Q/K/V projections of chunk j+1 and the output projection of earlier
chunks dribble between attention groups to keep the PE dense at 2.4 GHz.
S^T and exp are trimmed to the causal region at 128-column granularity.
"""

import numpy as np
from contextlib import ExitStack

import ml_dtypes

import concourse.bass as bass
import concourse.tile as tile
from concourse import bacc, mybir

F32 = mybir.dt.float32
BF16 = mybir.dt.bfloat16
AF = mybir.ActivationFunctionType

B, L, D = 4, 2048, 1024
N_HEAD, DH, DV = 16, 64, 64
N_CORES = 8
HPC = N_HEAD // 2          # heads per core (8)
OC = HPC * DH              # per-core projection width (512)
NHP = HPC // 2             # head-pairs per core (4)
NCH = L // 512             # q-chunks (4)
NLT = L // 128             # l-tiles (16)


def build_nc():
    nc = bacc.Bacc("TRN2", target_bir_lowering=False, debug=False,
                   num_devices=N_CORES)

    xt = nc.dram_tensor("xt", [128, 8, L], BF16, kind="ExternalInput").ap()
    wq = nc.dram_tensor("wq", [128, 8, OC], BF16, kind="ExternalInput").ap()
    wk = nc.dram_tensor("wk", [128, 8, OC], BF16, kind="ExternalInput").ap()
    wv = nc.dram_tensor("wv", [128, 8, OC], BF16, kind="ExternalInput").ap()
    wo = nc.dram_tensor("wo", [128, 4, D], BF16, kind="ExternalInput").ap()
    out = nc.dram_tensor("out", [L, D], F32, kind="ExternalOutput").ap()

    with tile.TileContext(nc) as tc, ExitStack() as ctx:
        top = ctx.enter_context(tc.tile_pool(name="top", bufs=1))
        psP = ctx.enter_context(tc.tile_pool(name="psP", bufs=2, space="PSUM"))
        psS = ctx.enter_context(tc.tile_pool(name="psS", bufs=2, space="PSUM"))
        psC = ctx.enter_context(tc.tile_pool(name="psC", bufs=2, space="PSUM"))
        phb = ctx.enter_context(tc.tile_pool(name="phb", bufs=2))
        pho = ctx.enter_context(tc.tile_pool(name="pho", bufs=3))

        xts = top.tile([128, 8, L], BF16)
        wqs = top.tile([128, 8, OC], BF16)
        wks = top.tile([128, 8, OC], BF16)
        wvs = top.tile([128, 8, OC], BF16)
        wos = top.tile([128, 4, D], BF16)
        qt = top.tile([128, NHP, L], BF16)
        kt = top.tile([128, NHP, L], BF16)
        # V: [128(k), ltile, head, 65] - col 64 is ones (softmax denominator)
        vt = top.tile([128, NLT, HPC, DV + 1], BF16)
        ct = top.tile([128, NHP, L], BF16)        # normalized ctx^T
        trilf = top.tile([128, 128], F32)
        tril = top.tile([128, 128], BF16)
        ones = top.tile([128, 1], BF16)

        # input DMAs; x chunked so chunk-0 projections can start early
        nc.sync.dma_start(out=wqs, in_=wq)
        nc.sync.dma_start(out=wks, in_=wk)
        nc.sync.dma_start(out=xts[:, :, 0:512], in_=xt[:, :, 0:512])
        nc.sync.dma_start(out=wvs, in_=wv)
        for c in range(1, NCH):
            nc.sync.dma_start(out=xts[:, :, c * 512:(c + 1) * 512],
                              in_=xt[:, :, c * 512:(c + 1) * 512])
        nc.sync.dma_start(out=wos, in_=wo)

        nc.vector.memset(ones, 1.0)
        nc.vector.tensor_copy(
            vt[:, :, :, DV:DV + 1].rearrange("p t h c -> p (t h) c"),
            ones.broadcast_to((128, NLT * HPC, 1)))
        # causal keep-mask for S^T diag blocks: tril[k, q] = 1.0 iff q >= k
        nc.gpsimd.memset(trilf, 0.0)
        nc.gpsimd.affine_select(
            out=trilf, in_=trilf, compare_op=mybir.AluOpType.is_gt,
            fill=1.0, base=0, pattern=[[-1, 128]], channel_multiplier=1)
        nc.vector.tensor_copy(tril, trilf)

        # ---------------- projection / output units ----------------
        # Units are emitted in halves (4 matmuls each) so they dribble
        # finely between attention groups. A half-open psum pair is always
        # closed by the immediately-following unit in the queue, so no
        # other psP allocation can interpose between A and B halves.
        pend = {}

        def proj_half(key, c, hp, half, wsrc, dst):
            def run():
                if half == 0:
                    pend[key] = psP.tile([128, 512], F32, tag="pp",
                                         name="pp")
                pp = pend[key]
                for d in range(4 * half, 4 * half + 4):
                    nc.tensor.matmul(pp, wsrc[:, d, hp * 128:(hp + 1) * 128],
                                     xts[:, d, c * 512:(c + 1) * 512],
                                     start=(d == 0), stop=(d == 7))
                if half == 1:
                    nc.vector.tensor_copy(dst[:, hp, c * 512:(c + 1) * 512],
                                          pp)
                    del pend[key]
            return run

        def v_half(lt, half):
            def run():
                key = ("v", lt)
                if half == 0:
                    pend[key] = psP.tile([128, 512], F32, tag="pp",
                                         name="pp")
                pp = pend[key]
                for d in range(4 * half, 4 * half + 4):
                    nc.tensor.matmul(pp, xts[:, d, lt * 128:(lt + 1) * 128],
                                     wvs[:, d, :], start=(d == 0),
                                     stop=(d == 7))
                if half == 1:
                    nc.vector.tensor_copy(
                        vt[:, lt, :, 0:DV],
                        pp.rearrange("p (h v) -> p h v", h=HPC))
                    del pend[key]
            return run

        ost_map = {}

        def o_half(lt, n, half):
            def run():
                key = ("o", lt, n)
                if half == 0:
                    if n == 0:
                        ost_map[lt] = pho.tile([128, D], F32, tag="ost",
                                               name="ost")
                    pend[key] = psP.tile([128, 512], F32, tag="pp",
                                         name="pp")
                pp = pend[key]
                for v in range(2 * half, 2 * half + 2):
                    nc.tensor.matmul(pp, ct[:, v, lt * 128:(lt + 1) * 128],
                                     wos[:, v, n * 512:(n + 1) * 512],
                                     start=(v == 0), stop=(v == 3))
                if half == 1:
                    ost = ost_map[lt]
                    nc.vector.tensor_copy(ost[:, n * 512:(n + 1) * 512], pp)
                    del pend[key]
                    if n == 1:
                        nc.sync.dma_start(
                            out=out[lt * 128:(lt + 1) * 128, :], in_=ost)
                        del ost_map[lt]
            return run

        def qk_halves(c, hp, wsrc, dst):
            tag = ("q" if dst is qt else "k", c, hp)
            return [proj_half(tag, c, hp, 0, wsrc, dst),
                    proj_half(tag, c, hp, 1, wsrc, dst)]

        def heater():
            # keeps the HAM clock gate at 2.4GHz when no real unit is ready
            pp = psP.tile([128, 512], F32, tag="pp", name="ppd")
            nc.tensor.matmul(pp[0:DV + 1, :], vt[:, 0, 0, :],
                             qt[:, 0, 0:512], start=True, stop=True)

        # ---------------- attention for one (head-pair, q-chunk) ----------
        def attention(hp, j, units):
            n_g = 2 * (j + 1)
            pctxs = {h: psC.tile([DV + 1, 512], F32, tag="pctx",
                                 name=f"pctx{h}") for h in range(2)}
            prevs = {0: None, 1: None}
            for g in range(n_g + 1):
                for h in range(2):
                    po = 64 * h
                    pexp = None
                    if g < n_g:
                        psc = psS.tile([128, 2, 512], F32, tag="psc",
                                       name=f"psc{h}")
                        pexp = phb.tile([128, 2, 512], BF16, tag="pexp",
                                        bufs=4, name=f"pexp{h}")
                        c0s = []
                        for r2 in range(2):
                            kt_i = 2 * g + r2
                            r = kt_i - 4 * j
                            c0 = 128 * r if r > 0 else 0
                            c0s.append(c0)
                            nc.tensor.matmul(
                                psc[:, r2, c0:512],
                                kt[po:po + DH, hp,
                                   kt_i * 128:(kt_i + 1) * 128],
                                qt[po:po + DH, hp,
                                   j * 512 + c0:(j + 1) * 512],
                                start=True, stop=True)
                        cm = min(c0s)
                        nc.scalar.activation(
                            pexp[:, :, cm:512], psc[:, :, cm:512],
                            AF.Exp, scale=0.125)
                        # mask the causal diagonal blocks right after exp
                        for r2 in range(2):
                            r = 2 * g + r2 - 4 * j
                            if r >= 0:
                                nc.vector.tensor_mul(
                                    pexp[:, r2, r * 128:(r + 1) * 128],
                                    pexp[:, r2, r * 128:(r + 1) * 128], tril)
                    # filler between the S pair and the PV pair: gives every
                    # stationary load a full matmul to hide under
                    if units:
                        units.pop(0)()
                    else:
                        heater()
                    if prevs[h] is not None:
                        pg, ppexp = prevs[h]
                        H = 2 * hp + h
                        for r2 in range(2):
                            kt_i = 2 * pg + r2
                            r = kt_i - 4 * j
                            c0 = 128 * r if r > 0 else 0
                            nc.tensor.matmul(
                                pctxs[h][:, c0:512],
                                vt[:, kt_i, H, :],
                                ppexp[:, r2, c0:512],
                                start=(kt_i == 0), stop=(kt_i == 4 * j + 3))
                    prevs[h] = (g, pexp) if g < n_g else None
            for h in range(2):
                po = 64 * h
                rs = phb.tile([1, 512], F32, tag="rs", name="rs")
                nc.vector.tensor_copy(rs, pctxs[h][DV:DV + 1, :])
                inv = phb.tile([1, 512], F32, tag="inv", name="inv")
                nc.vector.reciprocal_approx_fast(out=inv, in_=rs)
                bc = phb.tile([64, 512], F32, tag="bc", name="bc")
                nc.gpsimd.partition_broadcast(out_ap=bc, in_ap=inv)
                nc.vector.tensor_mul(
                    ct[po:po + DV, hp, j * 512:(j + 1) * 512],
                    pctxs[h][0:DV, :], bc)

        # ---------------- schedule ----------------
        # prologue: just enough of chunk 0 for attention(hp0) to start
        for u in qk_halves(0, 0, wqs, qt):
            u()
        for u in qk_halves(0, 0, wks, kt):
            u()
        for lt in range(4):
            v_half(lt, 0)()
            v_half(lt, 1)()

        for j in range(NCH):
            units = []
            if j == 0:
                for hp in range(1, NHP):
                    units += qk_halves(0, hp, wqs, qt)
                    units += qk_halves(0, hp, wks, kt)
                for lt in range(4, 8):
                    units += [v_half(lt, 0), v_half(lt, 1)]
                for hp in range(NHP):
                    units += qk_halves(1, hp, wqs, qt)
                for hp in range(NHP):
                    units += qk_halves(1, hp, wks, kt)
            elif j + 1 < NCH:
                for hp in range(NHP):
                    units += qk_halves(j + 1, hp, wqs, qt)
                    units += qk_halves(j + 1, hp, wks, kt)
                for lt in range(4 * (j + 1), 4 * (j + 2)):
                    units += [v_half(lt, 0), v_half(lt, 1)]
            if j >= 2:
                # O(j-2) late so the heaters land in the ACT-paced tail
                for lt in range(4 * (j - 2), 4 * (j - 1)):
                    for n in range(2):
                        units += [o_half(lt, n, 0), o_half(lt, n, 1)]
            if j == 3:
                for lt in range(8, 12):
                    for n in range(2):
                        units += [o_half(lt, n, 0), o_half(lt, n, 1)]
            for hp in range(NHP):
                attention(hp, j, units)
            while units:
                units.pop(0)()

        for lt in range(12, 16):
            for n in range(2):
                o_half(lt, n, 0)()
                o_half(lt, n, 1)()

    nc.compile()
    return nc


def make_in_maps(x, Wq, Wk, Wv, Wo):
    bf = ml_dtypes.bfloat16
    in_maps = []
    for c in range(N_CORES):
        b, g = c // 2, c % 2
        xtb = np.ascontiguousarray(
            x[b].T.reshape(8, 128, L).transpose(1, 0, 2)).astype(bf)
        wqg = np.ascontiguousarray(
            Wq[:, g * OC:(g + 1) * OC].reshape(8, 128, OC)
            .transpose(1, 0, 2)).astype(bf)
        wkg = np.ascontiguousarray(
            Wk[:, g * OC:(g + 1) * OC].reshape(8, 128, OC)
            .transpose(1, 0, 2)).astype(bf)
        wvg = np.ascontiguousarray(
            Wv[:, g * OC:(g + 1) * OC].reshape(8, 128, OC)
            .transpose(1, 0, 2)).astype(bf)
        wog = np.ascontiguousarray(
            Wo[g * OC:(g + 1) * OC, :].reshape(4, 128, D)
            .transpose(1, 0, 2)).astype(bf)
        in_maps.append({"xt": xtb, "wq": wqg, "wk": wkg, "wv": wvg,
                        "wo": wog})
    return in_maps


_NC_CACHE = {}


def _get_nc():
    if "nc" not in _NC_CACHE:
        _NC_CACHE["nc"] = build_nc()
    return _NC_CACHE["nc"]


def _numpy_fallback(x, Wq, Wk, Wv, Wo, bo, mask):
    Bsz, Lq, _ = x.shape
    Q = (x @ Wq).reshape(Bsz, Lq, N_HEAD, DH).transpose(0, 2, 1, 3)
    K = (x @ Wk).reshape(Bsz, Lq, N_HEAD, DH).transpose(0, 2, 1, 3)
    V = (x @ Wv).reshape(Bsz, Lq, N_HEAD, DV).transpose(0, 2, 1, 3)
    s = np.einsum("bhqd,bhkd->bhqk", Q, K) / np.sqrt(np.float32(DH))
    s = np.where(mask, s, -np.inf)
    s = s - s.max(axis=-1, keepdims=True)
    p = np.exp(s)
    p /= p.sum(axis=-1, keepdims=True)
    ctxv = np.einsum("bhqk,bhkv->bhqv", p, V)
    ctxv = ctxv.transpose(0, 2, 1, 3).reshape(Bsz, Lq, N_HEAD * DV)
    return (ctxv @ Wo + bo).astype(np.float32)


def run_on_hw(in_maps, trace=False):
    from concourse.bass_utils import run_bass_kernel_spmd
    nc = _get_nc()
    return run_bass_kernel_spmd(nc, in_maps, list(range(N_CORES)),
                                trace=trace)


def kernel(x, Wq, Wk, Wv, Wo, bo, mask, _trace=False, _results=None):
    x = np.asarray(x, dtype=np.float32)
    Wq = np.asarray(Wq, dtype=np.float32)
    Wk = np.asarray(Wk, dtype=np.float32)
    Wv = np.asarray(Wv, dtype=np.float32)
    Wo = np.asarray(Wo, dtype=np.float32)
    bo = np.asarray(bo, dtype=np.float32)
    mask_np = np.asarray(mask).reshape(mask.shape[-2], mask.shape[-1])

    causal = bool(np.array_equal(
        mask_np, np.tril(np.ones((L, L), dtype=bool))))
    if not causal or x.shape != (B, L, D):
        return _numpy_fallback(np.asarray(x), Wq, Wk, Wv, Wo, bo,
                               np.asarray(mask))

    res = run_on_hw(make_in_maps(x, Wq, Wk, Wv, Wo), trace=_trace)
    if _results is not None:
        _results.append(res)
    out = np.empty((B, L, D), dtype=np.float32)
    for b in range(B):
        out[b] = res.results[2 * b]["out"] + res.results[2 * b + 1]["out"] + bo
    return out


# revision 8
# speedup vs baseline: 1.3995x; 1.0493x over previous
"""Causal multi-head attention Trainium2 kernel (8 NeuronCores).

Problem: B=4, L=2048, D=1024, 16 heads x (dh=64, dv=64), causal mask.
Sharding: data-parallel over batch (4) x tensor-parallel over heads (2 groups
of 8). Core c handles batch c//2, head-group c%2. Each core computes its
partial output projection (ctx_g @ Wo_g); the host sums the two head-group
partials per batch and adds the bias.

v3: bf16 everywhere on SBUF (PSUM accumulates f32), x pre-transposed on the
host into [128, 8, L] d-major layout so the kernel has no PE transposes.
Loop order is q-chunk-outer: for each 512-query chunk j the four head-pairs
run flash-style attention (S^T = K@Q^T per 128-k-tile, exp on ACT with the
1/8 scale folded in, tril mask on the diagonal tiles, PV accumulation with
a ones-column of V giving the softmax denominator in PSUM row 64), while
the Q/K/V projections of chunk j+1 and the output projection of earlier
chunks dribble between attention groups to keep the PE dense at 2.4 GHz.
S^T and exp are trimmed to the causal region at 128-column granularity.
"""

import numpy as np
from contextlib import ExitStack

import ml_dtypes

import concourse.bass as bass
import concourse.tile as tile
from concourse import bacc, mybir

F32 = mybir.dt.float32
BF16 = mybir.dt.bfloat16
AF = mybir.ActivationFunctionType

B, L, D = 4, 2048, 1024
N_HEAD, DH, DV = 16, 64, 64
N_CORES = 8
HPC = N_HEAD // 2          # heads per core (8)
OC = HPC * DH              # per-core projection width (512)
NHP = HPC // 2             # head-pairs per core (4)
NCH = L // 512             # q-chunks (4)
NLT = L // 128             # l-tiles (16)


def build_nc():
    nc = bacc.Bacc("TRN2", target_bir_lowering=False, debug=False,
                   num_devices=N_CORES)

    xt = nc.dram_tensor("xt", [128, 8, L], BF16, kind="ExternalInput").ap()
    wq = nc.dram_tensor("wq", [128, 8, OC], BF16, kind="ExternalInput").ap()
    wk = nc.dram_tensor("wk", [128, 8, OC], BF16, kind="ExternalInput").ap()
    wv = nc.dram_tensor("wv", [128, 8, OC], BF16, kind="ExternalInput").ap()
    wo = nc.dram_tensor("wo", [128, 4, D], BF16, kind="ExternalInput").ap()
    out = nc.dram_tensor("out", [L, D], F32, kind="ExternalOutput").ap()

    with tile.TileContext(nc) as tc, ExitStack() as ctx:
        top = ctx.enter_context(tc.tile_pool(name="top", bufs=1))
        psP = ctx.enter_context(tc.tile_pool(name="psP", bufs=2, space="PSUM"))
        psS = ctx.enter_context(tc.tile_pool(name="psS", bufs=2, space="PSUM"))
        psC = ctx.enter_context(tc.tile_pool(name="psC", bufs=2, space="PSUM"))
        phb = ctx.enter_context(tc.tile_pool(name="phb", bufs=2))
        pho = ctx.enter_context(tc.tile_pool(name="pho", bufs=3))

        xts = top.tile([128, 8, L], BF16)
        wqs = top.tile([128, 8, OC], BF16)
        wks = top.tile([128, 8, OC], BF16)
        wvs = top.tile([128, 8, OC], BF16)
        wos = top.tile([128, 4, D], BF16)
        qt = top.tile([128, NHP, L], BF16)
        kt = top.tile([128, NHP, L], BF16)
        # V: [128(k), ltile, head, 65] - col 64 is ones (softmax denominator)
        vt = top.tile([128, NLT, HPC, DV + 1], BF16)
        ct = top.tile([128, NHP, L], BF16)        # normalized ctx^T
        trilf = top.tile([128, 128], F32)
        tril = top.tile([128, 128], BF16)
        ones = top.tile([128, 1], BF16)

        # input DMAs; x chunked so chunk-0 projections can start early
        nc.sync.dma_start(out=wqs, in_=wq)
        nc.sync.dma_start(out=wks, in_=wk)
        nc.sync.dma_start(out=xts[:, :, 0:512], in_=xt[:, :, 0:512])
        nc.sync.dma_start(out=wvs, in_=wv)
        for c in range(1, NCH):
            nc.sync.dma_start(out=xts[:, :, c * 512:(c + 1) * 512],
                              in_=xt[:, :, c * 512:(c + 1) * 512])
        nc.sync.dma_start(out=wos, in_=wo)

        nc.vector.memset(ones, 1.0)
        nc.vector.tensor_copy(
            vt[:, :, :, DV:DV + 1].rearrange("p t h c -> p (t h) c"),
            ones.broadcast_to((128, NLT * HPC, 1)))
        # causal keep-mask for S^T diag blocks: tril[k, q] = 1.0 iff q >= k
        nc.gpsimd.memset(trilf, 0.0)
        nc.gpsimd.affine_select(
            out=trilf, in_=trilf, compare_op=mybir.AluOpType.is_gt,
            fill=1.0, base=0, pattern=[[-1, 128]], channel_multiplier=1)
        nc.vector.tensor_copy(tril, trilf)

        # ---------------- projection / output units ----------------
        def qk_unit(c, hp, wsrc, dst):
            def run():
                pp = psP.tile([128, 512], F32, tag="pp", name="pp")
                for d in range(8):
                    nc.tensor.matmul(pp, wsrc[:, d, hp * 128:(hp + 1) * 128],
                                     xts[:, d, c * 512:(c + 1) * 512],
                                     start=(d == 0), stop=(d == 7))
                nc.vector.tensor_copy(dst[:, hp, c * 512:(c + 1) * 512], pp)
            return run

        def v_unit(lt):
            def run():
                pp = psP.tile([128, 512], F32, tag="pp", name="pp")
                for d in range(8):
                    nc.tensor.matmul(pp, xts[:, d, lt * 128:(lt + 1) * 128],
                                     wvs[:, d, :], start=(d == 0),
                                     stop=(d == 7))
                nc.vector.tensor_copy(
                    vt[:, lt, :, 0:DV],
                    pp.rearrange("p (h v) -> p h v", h=HPC))
            return run

        ost_map = {}

        def o_unit(lt, n):
            def run():
                if n == 0:
                    ost_map[lt] = pho.tile([128, D], F32, tag="ost",
                                           name="ost")
                ost = ost_map[lt]
                pp = psP.tile([128, 512], F32, tag="pp", name="pp")
                for v in range(4):
                    nc.tensor.matmul(pp, ct[:, v, lt * 128:(lt + 1) * 128],
                                     wos[:, v, n * 512:(n + 1) * 512],
                                     start=(v == 0), stop=(v == 3))
                nc.vector.tensor_copy(ost[:, n * 512:(n + 1) * 512], pp)
                if n == 1:
                    nc.sync.dma_start(out=out[lt * 128:(lt + 1) * 128, :],
                                      in_=ost)
                    del ost_map[lt]
            return run

        # ---------------- attention for one (head-pair, q-chunk) ----------
        def attention(hp, j, units):
            n_g = 2 * (j + 1)
            pctxs = {h: psC.tile([DV + 1, 512], F32, tag="pctx",
                                 name=f"pctx{h}") for h in range(2)}
            hist = {}
            for g in range(n_g + 1):
                cur = {}
                if g < n_g:
                    for h in range(2):
                        po = 64 * h
                        psc = psS.tile([128, 2, 512], F32, tag="psc",
                                       name=f"psc{h}")
                        pexp = phb.tile([128, 2, 512], BF16, tag="pexp",
                                        bufs=4, name=f"pexp{h}")
                        c0s = []
                        for r2 in range(2):
                            kt_i = 2 * g + r2
                            r = kt_i - 4 * j
                            c0 = 128 * r if r > 0 else 0
                            c0s.append(c0)
                            nc.tensor.matmul(
                                psc[:, r2, c0:512],
                                kt[po:po + DH, hp,
                                   kt_i * 128:(kt_i + 1) * 128],
                                qt[po:po + DH, hp,
                                   j * 512 + c0:(j + 1) * 512],
                                start=True, stop=True)
                        cm = min(c0s)
                        nc.scalar.activation(
                            pexp[:, :, cm:512], psc[:, :, cm:512],
                            AF.Exp, scale=0.125)
                        # mask the causal diagonal blocks right after exp
                        for r2 in range(2):
                            r = 2 * g + r2 - 4 * j
                            if r >= 0:
                                nc.vector.tensor_mul(
                                    pexp[:, r2, r * 128:(r + 1) * 128],
                                    pexp[:, r2, r * 128:(r + 1) * 128],
                                    tril)
                        cur[h] = pexp
                # one filler unit per group, between the S and PV quads, so
                # every stationary load hides under a full matmul
                if g < n_g:
                    hist[g] = cur
                if units:
                    units.pop(0)()
                pg = g - 1
                if pg in hist:
                    pex = hist.pop(pg)
                    for h in range(2):
                        H = 2 * hp + h
                        for r2 in range(2):
                            kt_i = 2 * pg + r2
                            r = kt_i - 4 * j
                            c0 = 128 * r if r > 0 else 0
                            nc.tensor.matmul(
                                pctxs[h][:, c0:512],
                                vt[:, kt_i, H, :],
                                pex[h][:, r2, c0:512],
                                start=(kt_i == 0), stop=(kt_i == 4 * j + 3))
            for h in range(2):
                po = 64 * h
                rs = phb.tile([1, 512], F32, tag="rs", name="rs")
                nc.vector.tensor_copy(rs, pctxs[h][DV:DV + 1, :])
                inv = phb.tile([1, 512], F32, tag="inv", name="inv")
                nc.vector.reciprocal_approx_fast(out=inv, in_=rs)
                bc = phb.tile([64, 512], F32, tag="bc", name="bc")
                nc.gpsimd.partition_broadcast(out_ap=bc, in_ap=inv)
                nc.vector.tensor_mul(
                    ct[po:po + DV, hp, j * 512:(j + 1) * 512],
                    pctxs[h][0:DV, :], bc)

        # ---------------- schedule ----------------
        # prologue: just enough of chunk 0 for attention(hp0) to start
        qk_unit(0, 0, wqs, qt)()
        qk_unit(0, 0, wks, kt)()
        for lt in range(4):
            v_unit(lt)()

        for j in range(NCH):
            units = []
            if j == 0:
                for hp in range(1, NHP):
                    units.append(qk_unit(0, hp, wqs, qt))
                    units.append(qk_unit(0, hp, wks, kt))
                for hp in range(NHP):
                    units.append(qk_unit(1, hp, wqs, qt))
                for hp in range(NHP):
                    units.append(qk_unit(1, hp, wks, kt))
                for lt in range(4, 8):
                    units.append(v_unit(lt))
            elif j + 1 < NCH:
                for hp in range(NHP):
                    units.append(qk_unit(j + 1, hp, wqs, qt))
                    units.append(qk_unit(j + 1, hp, wks, kt))
                for lt in range(4 * (j + 1), 4 * (j + 2)):
                    units.append(v_unit(lt))
            if j >= 2:
                # O(j-2) late so drains land in the attention-heavy tail
                for lt in range(4 * (j - 2), 4 * (j - 1)):
                    units.append(o_unit(lt, 0))
                    units.append(o_unit(lt, 1))
            for hp in range(NHP):
                attention(hp, j, units)
            while units:
                units.pop(0)()

        # O(2) has no dependence on the last normalize, so interleaving it
        # with O(3) fills the PE while the final softmax normalize drains
        for i in range(4):
            for n in range(2):
                o_unit(8 + i, n)()
            for n in range(2):
                o_unit(12 + i, n)()

    nc.compile()
    return nc


def make_in_maps(x, Wq, Wk, Wv, Wo):
    bf = ml_dtypes.bfloat16
    in_maps = []
    for c in range(N_CORES):
        b, g = c // 2, c % 2
        xtb = np.ascontiguousarray(
            x[b].T.reshape(8, 128, L).transpose(1, 0, 2)).astype(bf)
        wqg = np.ascontiguousarray(
            Wq[:, g * OC:(g + 1) * OC].reshape(8, 128, OC)
            .transpose(1, 0, 2)).astype(bf)
        wkg = np.ascontiguousarray(
            Wk[:, g * OC:(g + 1) * OC].reshape(8, 128, OC)
            .transpose(1, 0, 2)).astype(bf)
        wvg = np.ascontiguousarray(
            Wv[:, g * OC:(g + 1) * OC].reshape(8, 128, OC)
            .transpose(1, 0, 2)).astype(bf)
        wog = np.ascontiguousarray(
            Wo[g * OC:(g + 1) * OC, :].reshape(4, 128, D)
            .transpose(1, 0, 2)).astype(bf)
        in_maps.append({"xt": xtb, "wq": wqg, "wk": wkg, "wv": wvg,
                        "wo": wog})
    return in_maps


_NC_CACHE = {}


def _get_nc():
    if "nc" not in _NC_CACHE:
        _NC_CACHE["nc"] = build_nc()
    return _NC_CACHE["nc"]


def _numpy_fallback(x, Wq, Wk, Wv, Wo, bo, mask):
    Bsz, Lq, _ = x.shape
    Q = (x @ Wq).reshape(Bsz, Lq, N_HEAD, DH).transpose(0, 2, 1, 3)
    K = (x @ Wk).reshape(Bsz, Lq, N_HEAD, DH).transpose(0, 2, 1, 3)
    V = (x @ Wv).reshape(Bsz, Lq, N_HEAD, DV).transpose(0, 2, 1, 3)
    s = np.einsum("bhqd,bhkd->bhqk", Q, K) / np.sqrt(np.float32(DH))
    s = np.where(mask, s, -np.inf)
    s = s - s.max(axis=-1, keepdims=True)
    p = np.exp(s)
    p /= p.sum(axis=-1, keepdims=True)
    ctxv = np.einsum("bhqk,bhkv->bhqv", p, V)
    ctxv = ctxv.transpose(0, 2, 1, 3).reshape(Bsz, Lq, N_HEAD * DV)
    return (ctxv @ Wo + bo).astype(np.float32)


def run_on_hw(in_maps, trace=False):
    from concourse.bass_utils import run_bass_kernel_spmd
    nc = _get_nc()
    return run_bass_kernel_spmd(nc, in_maps, list(range(N_CORES)),
                                trace=trace)


def kernel(x, Wq, Wk, Wv, Wo, bo, mask, _trace=False, _results=None):
    x = np.asarray(x, dtype=np.float32)
    Wq = np.asarray(Wq, dtype=np.float32)
    Wk = np.asarray(Wk, dtype=np.float32)
    Wv = np.asarray(Wv, dtype=np.float32)
    Wo = np.asarray(Wo, dtype=np.float32)
    bo = np.asarray(bo, dtype=np.float32)
    mask_np = np.asarray(mask).reshape(mask.shape[-2], mask.shape[-1])

    causal = bool(np.array_equal(
        mask_np, np.tril(np.ones((L, L), dtype=bool))))
    if not causal or x.shape != (B, L, D):
        return _numpy_fallback(np.asarray(x), Wq, Wk, Wv, Wo, bo,
                               np.asarray(mask))

    res = run_on_hw(make_in_maps(x, Wq, Wk, Wv, Wo), trace=_trace)
    if _results is not None:
        _results.append(res)
    out = np.empty((B, L, D), dtype=np.float32)
    for b in range(B):
        out[b] = res.results[2 * b]["out"] + res.results[2 * b + 1]["out"] + bo
    return out
